# revision 26
# baseline (speedup 1.0000x reference)
"""DiM block (adaLN + Mamba selective scan + MLP) on 8 Trainium2 NeuronCores.

Sharding: core c = 2*b + i serves batch b; within a pair, core i owns
d_inner half i for the FULL sequence (the scan is fully local) and time
half i for the back-end (LN2/MLP).
Pair collectives: AllReduce of x_proj partials ([96,2048] f32, hidden
under the z-projection) and ReduceScatter of out_proj partials
([2,1024,1024] bf16 -> home time half).

Layout: channel-major — [channels -> partitions, time -> free dim].
Matmuls bf16/fp32r, weight-stationary ordering; scan decay path fp32;
value path bf16 (budget 2e-2).
"""
import sys
import numpy as np

sys.path.insert(0, "/opt/trn_rl_repo")

import concourse.bass as bass
import concourse.mybir as mybir
import concourse.tile as tile
from concourse import bacc
from concourse.bass_utils import run_bass_kernel_spmd
from concourse.masks import make_identity

import ml_dtypes

F32 = mybir.dt.float32
F32R = mybir.dt.float32r
BF16 = mybir.dt.bfloat16
AF = mybir.ActivationFunctionType
OP = mybir.AluOpType

P = 128
B, L, DIM = 4, 2048, 1024
D_STATE, D_CONV = 16, 4
D_INNER = 2048
DH = D_INNER // 2
DT_RANK = 64
MLP_HID = 4 * DIM
T = L
TH = L // 2
NC = 8
KD = DIM // P      # 8
KDH = DH // P      # 8
KH = MLP_HID // P  # 32
NT = T // 512      # 4
NTH = TH // 512    # 2
RG = [[0, 1], [2, 3], [4, 5], [6, 7]]
# scan mj groups sized to fit 3 ypsum [P,1024] tiles (6 PSUM banks),
# leaving one [P,1024] (2 banks) for the overlapped out_proj accumulation
SCAN_GROUPS = [(0, 1, 2), (3, 4, 5), (6, 7)]

_CACHE = {}


def _build():
    nc = bacc.Bacc("TRN2", target_bir_lowering=False, debug=False, num_devices=NC)
    ein = lambda n, s, d=F32: nc.dram_tensor(n, s, d, kind="ExternalInput")
    hsT = ein("hsT", (DIM, T), BF16)
    cond_c = ein("cond_c", (P, KD))
    w_ada = ein("w_ada", (DIM, 6 * DIM), BF16)
    ada_b = ein("ada_b", (1, 6 * DIM))
    nrm_w = ein("nrm_w", (P, KD))
    nrm_b = ein("nrm_b", (P, KD))
    w_inx = ein("w_inx", (DIM, DH), BF16)
    w_inz = ein("w_inz", (DIM, DH), BF16)
    conv_wt = ein("conv_wt", (P, KDH * D_CONV))
    conv_bt = ein("conv_bt", (P, KDH))
    w_xp = ein("w_xp", (DH, 96), BF16)
    w_dt = ein("w_dt", (P, DH), BF16)
    dt_bt = ein("dt_bt", (P, KDH))
    A_sc = ein("A_sc", (P, KDH * D_STATE))
    Dp_c = ein("Dp_c", (P, KDH))
    w_out = ein("w_out", (DH, DIM), BF16)        # my d-half rows only
    w_fc1 = ein("w_fc1", (DIM, MLP_HID), BF16)
    fc1_bt = ein("fc1_bt", (P, KH))
    w_fc2 = ein("w_fc2", (MLP_HID, DIM), BF16)
    fc2_bt = ein("fc2_bt", (P, KD))
    out_hs = nc.dram_tensor("out_hs", (DIM, TH), F32, kind="ExternalOutput")

    with tile.TileContext(nc) as tc, nc.allow_low_precision(
            reason="bf16 value path; rel-err budget 2e-2"):
        import contextlib
        ctx = contextlib.ExitStack()
        sing = ctx.enter_context(tc.tile_pool(name="sing", bufs=1))
        ws = ctx.enter_context(tc.tile_pool(name="ws", bufs=2))
        wk1 = ctx.enter_context(tc.tile_pool(name="wk1", bufs=1))
        wk2 = ctx.enter_context(tc.tile_pool(name="wk2", bufs=2))
        ga = ctx.enter_context(tc.tile_pool(name="ga", bufs=3))
        ps = ctx.enter_context(tc.tile_pool(name="ps", bufs=1, space="PSUM"))
        dpool = ctx.enter_context(tc.tile_pool(name="dram", bufs=1, space="DRAM"))

        def q(i, shape=(P, 1024), dt_=F32, name="q"):
            return ps.tile(list(shape), dt_, tag=f"q{i}", name=f"{name}{i}")

        # DRAM scratch
        md_dram = dpool.tile([1, 6 * DIM], F32)
        cc1_in = dpool.tile([96, T], F32)
        cc1_out = dpool.tile([96, T], F32)
        bc_dram = dpool.tile([2 * D_STATE, T], BF16)
        zs_dram = dpool.tile([P, KDH, T], BF16)
        cc2_in = dpool.tile([2, P, KD, TH], BF16)   # out_proj partials
        cc2_out = dpool.tile([P, KD, TH], BF16)     # reduced home half

        # ---- small persistents ----
        cond_sb = sing.tile([P, KD], F32)
        nc.sync.dma_start(cond_sb[:], cond_c[:])
        scond = sing.tile([P, KD], BF16)
        nc.scalar.activation(scond[:], cond_sb[:], AF.Silu)
        nwb_sb = sing.tile([P, 2 * KD], F32)
        nc.sync.dma_start(nwb_sb[:, 0:KD], nrm_w[:])
        nc.sync.dma_start(nwb_sb[:, KD:2 * KD], nrm_b[:])
        cwb_sb = sing.tile([P, KDH * D_CONV + KDH], F32)
        nc.sync.dma_start(cwb_sb[:, 0:KDH * D_CONV], conv_wt[:, :])
        nc.sync.dma_start(cwb_sb[:, KDH * D_CONV:], conv_bt[:])
        dtb_sb = sing.tile([P, KDH], F32)
        nc.sync.dma_start(dtb_sb[:], dt_bt[:])
        Asc_sb = sing.tile([P, KDH * D_STATE], F32)
        nc.sync.dma_start(Asc_sb[:], A_sc[:])
        Dp_sb = sing.tile([P, KDH], F32)
        nc.sync.dma_start(Dp_sb[:], Dp_c[:])
        fb_sb = sing.tile([P, KH + KD], F32)
        nc.sync.dma_start(fb_sb[:, 0:KH], fc1_bt[:])
        nc.sync.dma_start(fb_sb[:, KH:], fc2_bt[:])
        ident = sing.tile([P, P], BF16)
        make_identity(nc, ident[:])
        ones_bf = sing.tile([P, 1], BF16)
        nc.vector.memset(ones_bf[:], 1.0)
        ones_row = sing.tile([1, P], F32R)
        nc.vector.memset(ones_row[:].bitcast(F32), 1.0)
        eps_sb = sing.tile([1, 2], F32)
        nc.vector.memset(eps_sb[:, 0:1], 1e-5)
        nc.vector.memset(eps_sb[:, 1:2], 1e-6)

        # ---- adaLN mods (bf16) ----
        for fj in range(12):
            wada = ws.tile([P, KD, 512], BF16, tag="ws")
            nc.sync.dma_start(wada[:], w_ada[:, fj * 512:(fj + 1) * 512]
                              .rearrange("(k p) f -> p k f", p=P))
            mp = q(fj % 2, (1, 512), name="mp")
            for kj in range(KD):
                nc.tensor.matmul(mp[:], scond[:, kj:kj + 1], wada[:, kj],
                                 start=(kj == 0), stop=(kj == KD - 1))
            adab_c = ga.tile([1, 512], F32, tag="gR")
            nc.sync.dma_start(adab_c[:], ada_b[0:1, fj * 512:(fj + 1) * 512])
            mrow = ga.tile([1, 512], F32, tag="gR")
            nc.vector.tensor_tensor(mrow[:], mp[:], adab_c[:], OP.add)
            nc.sync.dma_start(md_dram[0:1, fj * 512:(fj + 1) * 512], mrow[:])
        modsT = sing.tile([P, 6 * KD], F32)
        nc.sync.dma_start(modsT[:, 0:2 * KD],
                          md_dram[0:1, 0:2 * DIM].rearrange("q (c p) -> (q p) c", p=P))
        sh_msa, sc_msa, gt_msa, sh_mlp, sc_mlp, gt_mlp = (
            modsT[:, k * KD:(k + 1) * KD] for k in range(6))
        a1 = sing.tile([P, 3 * KD], F32)   # [a1 | c1 | a2]
        tmp8 = sing.tile([P, KD], F32)
        nc.vector.tensor_scalar(tmp8[:], sc_msa, 1.0, None, OP.add)
        nc.vector.tensor_tensor(a1[:, 0:KD], nwb_sb[:, 0:KD], tmp8[:], OP.mult)
        nc.vector.tensor_tensor(a1[:, KD:2 * KD], nwb_sb[:, KD:2 * KD], tmp8[:], OP.mult)
        nc.vector.tensor_tensor(a1[:, KD:2 * KD], a1[:, KD:2 * KD], sh_msa, OP.add)
        nc.sync.dma_start(modsT[:, 2 * KD:],
                          md_dram[0:1, 2 * DIM:].rearrange("q (c p) -> (q p) c", p=P))
        nc.vector.tensor_scalar(a1[:, 2 * KD:3 * KD], sc_mlp, 1.0, None, OP.add)
        gt2 = sing.tile([P, KD], F32)
        nc.vector.tensor_tensor(gt2[:], gt_msa, gt_msa, OP.mult)
        gt_bf = sing.tile([P, 2 * KD], BF16)
        nc.vector.tensor_copy(gt_bf[:, 0:KD], gt_msa)
        nc.vector.tensor_copy(gt_bf[:, KD:2 * KD], gt2[:])

        # ---- LN1 + modulate, fused per 512-tile ----
        x_in = sing.tile([P, KD, T], BF16, tag="BIGA")
        for ft in range(NT):
            fs = slice(ft * 512, ft * 512 + 512)
            hst = ws.tile([P, KD, 512], BF16, tag="ws")
            nc.sync.dma_start(hst[:], hsT[:, fs].rearrange("(k p) t -> p k t", p=P))
            sp = q(0, (1, 512), name="sp")
            qp = q(1, (1, 512), name="qp")
            for kj in range(KD):
                nc.tensor.matmul(sp[:], ones_bf[:], hst[:, kj],
                                 start=(kj == 0), stop=(kj == KD - 1))
            for kj in range(KD):
                sq = ga.tile([P, 512], BF16, tag="gA")
                nc.scalar.activation(sq[:], hst[:, kj], AF.Square)
                nc.tensor.matmul(qp[:], ones_bf[:], sq[:],
                                 start=(kj == 0), stop=(kj == KD - 1))
            mu = ga.tile([1, 512], F32R, tag="gR")
            nc.vector.tensor_scalar(mu[:], sp[:], 1.0 / DIM, None, OP.mult)
            mq = ga.tile([1, 512], F32, tag="gR")
            nc.vector.tensor_tensor(mq[:], mu[:], mu[:], OP.mult)
            vt = ga.tile([1, 512], F32, tag="gR")
            nc.vector.scalar_tensor_tensor(vt[:], qp[:], 1.0 / DIM, mq[:],
                                           OP.mult, OP.subtract)
            nc.scalar.activation(vt[:], vt[:], AF.Sqrt, bias=eps_sb[:, 0:1])
            rs = ga.tile([1, 512], F32R, tag="gR")
            nc.vector.reciprocal(rs[:], vt[:])
            mr_pl = q(2, (P, 1024), name="mr_pl")
            nc.tensor.matmul(mr_pl[:, 0:512], ones_row[:], mu[:], start=True, stop=True)
            nc.tensor.matmul(mr_pl[:, 512:1024], ones_row[:], rs[:], start=True, stop=True)
            for kj in range(KD):
                t1 = ga.tile([P, 512], F32, tag="gA")
                nc.vector.tensor_tensor(t1[:], hst[:, kj], mr_pl[:, 0:512], OP.subtract)
                nc.vector.tensor_tensor(t1[:], t1[:], mr_pl[:, 512:1024], OP.mult)
                nc.scalar.activation(x_in[:, kj, fs], t1[:], AF.Identity,
                                     scale=a1[:, kj:kj + 1],
                                     bias=a1[:, KD + kj:KD + kj + 1])

        # ---- in_proj X + conv + silu -> xcv_bf; x_proj folded per mj ----
        xcv_bf = sing.tile([P, KDH, T], BF16, tag="XCVB")
        xdp01 = q(2, (96, 1024), name="xdp01")
        xdp23 = q(3, (96, 1024), name="xdp23")
        xdps = [xdp01[:, 0:512], xdp01[:, 512:1024],
                xdp23[:, 0:512], xdp23[:, 512:1024]]
        wxp_sb = sing.tile([P, KDH, 96], BF16)
        nc.sync.dma_start(wxp_sb[:], w_xp[:, :].rearrange("(k p) m -> p k m", p=P))
        for mj in range(KDH):
            wx = ws.tile([P, KD, P], BF16, tag="wsm", bufs=3)
            nc.sync.dma_start(wx[:], w_inx[:, mj * P:(mj + 1) * P]
                              .rearrange("(k p) m -> p k m", p=P))
            px0 = q(0, name="px")
            px1 = q(1, name="px")
            pxs = [px0, px1]
            for ft in range(NT):
                for kj in range(KD):
                    nc.tensor.matmul(pxs[ft // 2][:, (ft % 2) * 512:(ft % 2) * 512 + 512],
                                     wx[:, kj], x_in[:, kj, ft * 512:(ft + 1) * 512],
                                     start=(kj == 0), stop=(kj == KD - 1))
            xpad = wk1.tile([P, D_CONV - 1 + T], BF16, tag="xpad")
            nc.vector.memset(xpad[:, 0:D_CONV - 1], 0.0)
            for ft in range(NT):
                nc.scalar.activation(
                    xpad[:, D_CONV - 1 + ft * 512:D_CONV - 1 + (ft + 1) * 512],
                    pxs[ft // 2][:, (ft % 2) * 512:(ft % 2) * 512 + 512], AF.Copy)
            cvt = wk1.tile([P, T], BF16, tag="cvt")
            nc.vector.tensor_scalar(cvt[:], xpad[:, 0:T],
                                    cwb_sb[:, mj * D_CONV:mj * D_CONV + 1],
                                    None, OP.mult)
            for j in range(1, D_CONV):
                nc.vector.scalar_tensor_tensor(
                    cvt[:], xpad[:, j:j + T],
                    cwb_sb[:, mj * D_CONV + j:mj * D_CONV + j + 1], cvt[:],
                    OP.mult, OP.add)
            nc.scalar.activation(
                xcv_bf[:, mj], cvt[:], AF.Silu,
                bias=cwb_sb[:, KDH * D_CONV + mj:KDH * D_CONV + mj + 1])
            for ft in range(NT):
                nc.tensor.matmul(xdps[ft][:], wxp_sb[:, mj],
                                 xcv_bf[:, mj, ft * 512:(ft + 1) * 512],
                                 start=(mj == 0), stop=(mj == KDH - 1))

        # ---- x_proj partials -> DRAM + CC1: AllReduce ----
        for ft in range(NT):
            xdc = ga.tile([96, 512], F32, tag="gA")
            nc.vector.tensor_copy(xdc[:], xdps[ft][:])
            nc.sync.dma_start(cc1_in[:, ft * 512:(ft + 1) * 512], xdc[:])
        nc.gpsimd.collective_compute("AllReduce", OP.add, ins=[cc1_in[:]],
                                     outs=[cc1_out[:]], replica_groups=RG)

        # ---- dt (softplus) + u;  B/C rows -> bc_dram ----
        dtlow = wk1.tile([P, T], BF16, tag="cvt", name="dtlow")
        nc.vector.memset(dtlow[:], 0.0)
        nc.gpsimd.dma_start(dtlow[0:DT_RANK, :], cc1_out[0:DT_RANK, :])
        nc.gpsimd.dma_start(bc_dram[:], cc1_out[DT_RANK:96, :])
        wdt_sb = wk1.tile([P, DH], BF16, tag="xpad", name="wdt_sb")
        nc.sync.dma_start(wdt_sb[:], w_dt[:])
        one_col = sing.tile([P, 1], F32)
        nc.vector.memset(one_col[:], 1.0)
        dtu = sing.tile([P, 2 * KDH, T], BF16, tag="BIGA")   # [0:8]=dt, [8:16]=u

        def dt_for_mj(mj):
            # softplus(x+b) = ln(1 + exp(x+b)); exp scratch in the u-slot,
            # then u = dt * xcv overwrites it. Uses q3 (out_proj's tag, idle
            # during the tb0 scan) so it can interleave with scan groups.
            pdt = q(3, name="pd")
            for fh in range(2):
                for ft in (2 * fh, 2 * fh + 1):
                    nc.tensor.matmul(pdt[:, (ft % 2) * 512:(ft % 2) * 512 + 512],
                                     wdt_sb[:, mj * P:(mj + 1) * P],
                                     dtlow[:, ft * 512:(ft + 1) * 512],
                                     start=True, stop=True)
                for ft in (2 * fh, 2 * fh + 1):
                    nc.scalar.activation(
                        dtu[:, KDH + mj, ft * 512:(ft + 1) * 512],
                        pdt[:, (ft % 2) * 512:(ft % 2) * 512 + 512],
                        AF.Exp, bias=dtb_sb[:, mj:mj + 1])
                pdt = q(3, name="pd")
            nc.scalar.activation(dtu[:, mj], dtu[:, KDH + mj], AF.Ln, bias=one_col[:])
            nc.vector.tensor_tensor(dtu[:, KDH + mj], dtu[:, mj], xcv_bf[:, mj], OP.mult)

        # ---- in_proj Z (runs during AllReduce) -> silu -> zs_dram ----
        for mj in range(KDH):
            wz = ws.tile([P, KD, P], BF16, tag="wsm", bufs=3)
            nc.sync.dma_start(wz[:], w_inz[:, mj * P:(mj + 1) * P]
                              .rearrange("(k p) m -> p k m", p=P))
            pz0 = q(2 * (mj % 2), name="pz")
            pz1 = q(2 * (mj % 2) + 1, name="pz")
            pzs = [pz0, pz1]
            for ft in range(NT):
                for kj in range(KD):
                    nc.tensor.matmul(pzs[ft // 2][:, (ft % 2) * 512:(ft % 2) * 512 + 512],
                                     wz[:, kj], x_in[:, kj, ft * 512:(ft + 1) * 512],
                                     start=(kj == 0), stop=(kj == KD - 1))
            for ft in range(NT):
                zc = ga.tile([P, 512], BF16, tag="gA")
                nc.scalar.activation(zc[:],
                                     pzs[ft // 2][:, (ft % 2) * 512:(ft % 2) * 512 + 512],
                                     AF.Silu)
                nc.sync.dma_start(zs_dram[:, mj, ft * 512:(ft + 1) * 512], zc[:])

        # ---- scan + y + gate + out_proj partials (overlapped) ----
        hcol = sing.tile([P, KDH * D_STATE], F32)
        NBT = 2
        TB = T // NBT

        def out_proj_block(tb, ygated):
            # out partial for time block tb: [DIM, TB] = sum_mj w_out[mj] y[mj]
            for oj in range(KD):
                wo = ws.tile([P, KDH, P], BF16, tag="wsm", bufs=3)
                nc.sync.dma_start(wo[:], w_out[:, oj * P:(oj + 1) * P]
                                  .rearrange("(k p) m -> p k m", p=P))
                po = q(3, name="po")
                for kj in range(KDH):
                    for hh in range(2):
                        nc.tensor.matmul(
                            po[:, hh * 512:(hh + 1) * 512], wo[:, kj],
                            ygated[:, kj, hh * 512:(hh + 1) * 512],
                            start=(kj == 0), stop=(kj == KDH - 1))
                for hh in range(2):
                    oc = ga.tile([P, 512], BF16, tag="gA")
                    nc.scalar.activation(oc[:], po[:, hh * 512:(hh + 1) * 512], AF.Copy)
                    nc.sync.dma_start(
                        cc2_in[tb, :, oj, hh * 512:(hh + 1) * 512], oc[:])

        ym_rr = 0  # round-robin: every 3rd ym on Pool (gpsimd), rest on DVE
        for tb in range(NBT):
            ts_ = slice(tb * TB, (tb + 1) * TB)
            ygated = wk1.tile([P, KDH, TB], BF16, tag="YGD", name=f"ygated{tb}")
            for gi, grp in enumerate(SCAN_GROUPS):
                if tb == 0:
                    for mj in grp:
                        dt_for_mj(mj)
                ypsum = {mj: q(j, name="ypsum") for j, mj in enumerate(grp)}
                for mj in grp:
                    dpl = wk2.tile([P, TB], BF16, tag="bin", name="dpl")
                    nc.scalar.activation(dpl[:], xcv_bf[:, mj, ts_], AF.Identity,
                                         scale=Dp_sb[:, mj:mj + 1])
                    for hh in range(2):
                        nc.tensor.matmul(ypsum[mj][:, hh * 512:(hh + 1) * 512],
                                         ident[:], dpl[:, hh * 512:(hh + 1) * 512],
                                         start=True, stop=False)
                for n in range(D_STATE):
                    bpl = wk2.tile([P, TB], BF16, tag="bpl")
                    nc.sync.dma_start(bpl[:], bc_dram[n:n + 1, ts_].partition_broadcast(P))
                    cpl = wk2.tile([P, TB], BF16, tag="cpl")
                    nc.sync.dma_start(cpl[:], bc_dram[D_STATE + n:D_STATE + n + 1, ts_]
                                      .partition_broadcast(P))
                    for mj in grp:
                        da = wk2.tile([P, TB], F32, tag="da")
                        nc.scalar.activation(
                            da[:], dtu[:, mj, ts_], AF.Exp,
                            scale=Asc_sb[:, mj * D_STATE + n:mj * D_STATE + n + 1])
                        bin_ = wk2.tile([P, TB], BF16, tag="bin")
                        beng = nc.gpsimd if ym_rr % 5 == 3 else nc.vector
                        beng.tensor_tensor(bin_[:], dtu[:, KDH + mj, ts_], bpl[:],
                                           OP.mult)
                        h = wk2.tile([P, TB], BF16, tag="h", bufs=4)
                        init = (0.0 if tb == 0
                                else hcol[:, mj * D_STATE + n:mj * D_STATE + n + 1])
                        nc.vector.tensor_tensor_scan(h[:], da[:], bin_[:], init,
                                                     OP.mult, OP.add)
                        if tb < NBT - 1:
                            nc.scalar.activation(
                                hcol[:, mj * D_STATE + n:mj * D_STATE + n + 1],
                                h[:, TB - 1:TB], AF.Copy)
                        ym = wk2.tile([P, TB], BF16, tag="ym")
                        eng = nc.gpsimd if ym_rr % 5 in (0, 2) else nc.vector
                        ym_rr += 1
                        eng.tensor_tensor(ym[:], h[:], cpl[:], OP.mult)
                        for hh in range(2):
                            nc.tensor.matmul(ypsum[mj][:, hh * 512:(hh + 1) * 512],
                                             ident[:], ym[:, hh * 512:(hh + 1) * 512],
                                             start=False, stop=(n == D_STATE - 1))
                for mj in grp:
                    zl = wk2.tile([P, TB], BF16, tag="bpl", name="zl")
                    nc.sync.dma_start(zl[:], zs_dram[:, mj, ts_])
                    nc.vector.tensor_tensor(ygated[:, mj], ypsum[mj][:], zl[:], OP.mult)
            out_proj_block(tb, ygated)
        nc.gpsimd.collective_compute("ReduceScatter", OP.add, ins=[cc2_in[:]],
                                     outs=[cc2_out[:]], replica_groups=RG)

        # ---- gate_msa + LN2 + modulate (SBUF resident) ----
        hsm = sing.tile([P, KD, TH], F32, tag="XCVB")
        h2_bf = sing.tile([P, KD, TH], BF16, tag="ZSC")
        for ft in range(NTH):
            fs = slice(ft * 512, ft * 512 + 512)
            hmst = ws.tile([P, KD, 512], BF16, tag="ws")
            nc.sync.dma_start(hmst[:], cc2_out[:, :, fs])
            sp = q(0, (1, 512), name="sp2")
            qp = q(1, (1, 512), name="qp2")
            for kj in range(KD):
                nc.vector.tensor_scalar(hsm[:, kj, fs], hmst[:, kj],
                                        gt_msa[:, kj:kj + 1], None, OP.mult)
            # gated sums via gate columns as the stationary operand:
            # sum_t gate_p*x_pt and sum_t (gate_p*x_pt)^2 = gate^2 * x^2
            for kj in range(KD):
                nc.tensor.matmul(sp[:], gt_bf[:, kj:kj + 1], hmst[:, kj],
                                 start=(kj == 0), stop=(kj == KD - 1))
            for kj in range(KD):
                sq = ga.tile([P, 512], BF16, tag="gA")
                nc.scalar.activation(sq[:], hmst[:, kj], AF.Square)
                nc.tensor.matmul(qp[:], gt_bf[:, KD + kj:KD + kj + 1], sq[:],
                                 start=(kj == 0), stop=(kj == KD - 1))
            mu = ga.tile([1, 512], F32R, tag="gR")
            nc.vector.tensor_scalar(mu[:], sp[:], 1.0 / DIM, None, OP.mult)
            mq = ga.tile([1, 512], F32, tag="gR")
            nc.vector.tensor_tensor(mq[:], mu[:], mu[:], OP.mult)
            vt = ga.tile([1, 512], F32, tag="gR")
            nc.vector.scalar_tensor_tensor(vt[:], qp[:], 1.0 / DIM, mq[:],
                                           OP.mult, OP.subtract)
            nc.scalar.activation(vt[:], vt[:], AF.Sqrt, bias=eps_sb[:, 1:2])
            rs = ga.tile([1, 512], F32R, tag="gR")
            nc.vector.reciprocal(rs[:], vt[:])
            mr_pl = q(2, (P, 1024), name="mr_pl2")
            nc.tensor.matmul(mr_pl[:, 0:512], ones_row[:], mu[:], start=True, stop=True)
            nc.tensor.matmul(mr_pl[:, 512:1024], ones_row[:], rs[:], start=True, stop=True)
            for kj in range(KD):
                t1 = ga.tile([P, 512], F32, tag="gA")
                nc.vector.tensor_tensor(t1[:], hsm[:, kj, fs], mr_pl[:, 0:512],
                                        OP.subtract)
                nc.vector.tensor_tensor(t1[:], t1[:], mr_pl[:, 512:1024], OP.mult)
                nc.scalar.activation(h2_bf[:, kj, fs], t1[:], AF.Identity,
                                     scale=a1[:, 2 * KD + kj:2 * KD + kj + 1],
                                     bias=sh_mlp[:, kj:kj + 1])

        # ---- MLP (weight-stationary) ----
        hmlp = sing.tile([P, KH, TH], BF16, tag="BIGA")
        for mj in range(KH):
            wf1 = ws.tile([P, KD, P], BF16, tag="wsm", bufs=3)
            nc.sync.dma_start(wf1[:], w_fc1[:, mj * P:(mj + 1) * P]
                              .rearrange("(k p) m -> p k m", p=P))
            pm = q(mj % 2, name="pm")
            for kj in range(KD):
                for ft in range(NTH):
                    nc.tensor.matmul(pm[:, ft * 512:(ft + 1) * 512],
                                     wf1[:, kj], h2_bf[:, kj, ft * 512:(ft + 1) * 512],
                                     start=(kj == 0), stop=(kj == KD - 1))
            for ft in range(NTH):
                nc.scalar.activation(hmlp[:, mj, ft * 512:(ft + 1) * 512],
                                     pm[:, ft * 512:(ft + 1) * 512],
                                     AF.Gelu_apprx_tanh, bias=fb_sb[:, mj:mj + 1])
        for oj in range(KD):
            wf2 = ws.tile([P, KH, P], BF16, tag="ws")
            nc.sync.dma_start(wf2[:], w_fc2[:, oj * P:(oj + 1) * P]
                              .rearrange("(k p) m -> p k m", p=P))
            pf = q(2 + oj % 2, name="pf")
            for kj in range(KH):
                for ft in range(NTH):
                    nc.tensor.matmul(pf[:, ft * 512:(ft + 1) * 512],
                                     wf2[:, kj], hmlp[:, kj, ft * 512:(ft + 1) * 512],
                                     start=(kj == 0), stop=(kj == KH - 1))
            for ft in range(NTH):
                fs = slice(ft * 512, ft * 512 + 512)
                t1 = ga.tile([P, 512], F32, tag="gA")
                nc.vector.tensor_scalar(t1[:], pf[:, fs], fb_sb[:, KH + oj:KH + oj + 1],
                                        gt_mlp[:, oj:oj + 1], OP.add, OP.mult)
                nc.vector.tensor_tensor(t1[:], t1[:], hsm[:, oj, fs], OP.add)
                nc.sync.dma_start(
                    out_hs[:, :].rearrange("(k p) t -> p k t", p=P)[:, oj, fs], t1[:])
        ctx.close()
    nc.compile()
    return nc


def _prep_inputs(kw):
    f32 = np.float32
    bf16 = ml_dtypes.bfloat16
    g = lambda k: np.asarray(kw[k], f32)
    hs, cond = g("hidden_states"), g("cond")
    in_w = g("in_proj_w")
    A = -np.exp(g("A_log"))
    out_w = g("out_proj_w")

    def colchunks(v):
        return np.ascontiguousarray(v.reshape(-1, P).T)

    adaT = np.ascontiguousarray(g("ada_w").T).astype(bf16)
    fc1T = np.ascontiguousarray(g("fc1_w").T).astype(bf16)
    fc2T = np.ascontiguousarray(g("fc2_w").T).astype(bf16)
    in_maps = []
    for c in range(NC):
        b, i = c // 2, c % 2
        dsl = slice(i * DH, (i + 1) * DH)
        A_h = A[dsl]
        A_pack = np.zeros((P, KDH * D_STATE), f32)
        for mj in range(KDH):
            A_pack[:, mj * D_STATE:(mj + 1) * D_STATE] = A_h[mj * P:(mj + 1) * P]
        cw = g("conv_w")[dsl, 0, :].reshape(KDH, P, D_CONV).transpose(1, 0, 2)
        in_maps.append({
            "hsT": np.ascontiguousarray(hs[b].T).astype(bf16),
            "cond_c": colchunks(cond[b]),
            "w_ada": adaT,
            "ada_b": g("ada_b").reshape(1, -1),
            "nrm_w": colchunks(g("norm_w")),
            "nrm_b": colchunks(g("norm_b")),
            "w_inx": np.ascontiguousarray(in_w[dsl].T).astype(bf16),
            "w_inz": np.ascontiguousarray(in_w[D_INNER:][dsl].T).astype(bf16),
            "conv_wt": np.ascontiguousarray(cw.reshape(P, KDH * D_CONV)),
            "conv_bt": colchunks(g("conv_b")[dsl]),
            "w_xp": np.ascontiguousarray(g("x_proj_w")[:, dsl].T).astype(bf16),
            "w_dt": np.concatenate([g("dt_proj_w")[dsl].T,
                                    np.zeros((P - DT_RANK, DH), f32)], 0).astype(bf16),
            "dt_bt": colchunks(g("dt_proj_b")[dsl]),
            "A_sc": A_pack,
            "Dp_c": colchunks(g("Dp")[dsl]),
            "w_out": np.ascontiguousarray(out_w[:, dsl].T).astype(bf16),
            "w_fc1": fc1T,
            "fc1_bt": colchunks(g("fc1_b")),
            "w_fc2": fc2T,
            "fc2_bt": colchunks(g("fc2_b")),
        })
    return in_maps


def kernel(**inputs):
    if "nc" not in _CACHE:
        _CACHE["nc"] = _build()
    nc = _CACHE["nc"]
    in_maps = _prep_inputs(inputs)
    res = run_bass_kernel_spmd(nc, in_maps, list(range(NC)))
    hs_out = np.empty((B, L, DIM), np.float32)
    for c in range(NC):
        b, i = c // 2, c % 2
        hs_out[b, i * TH:(i + 1) * TH, :] = res.results[c]["out_hs"].T
    residual = np.asarray(inputs["hidden_states"], np.float32)
    return hs_out, residual


# revision 27
# speedup vs baseline: 1.8453x; 1.8453x over previous
"""DiM block (adaLN + Mamba selective scan + MLP) on 8 Trainium2 NeuronCores.

Sharding: core c = 2*b + i serves batch b; within a pair, core i owns
d_inner half i for the FULL sequence (the scan is fully local) and time
half i for the back-end (LN2/MLP).
Pair collectives: AllReduce of x_proj partials ([96,2048] f32, hidden
under the z-projection) and ReduceScatter of out_proj partials
([2,1024,1024] bf16 -> home time half).

Layout: channel-major — [channels -> partitions, time -> free dim].
Matmuls bf16/fp32r, weight-stationary ordering; scan decay path fp32;
value path bf16 (budget 2e-2).
"""
import sys
import numpy as np

sys.path.insert(0, "/opt/trn_rl_repo")

import concourse.bass as bass
import concourse.mybir as mybir
import concourse.tile as tile
from concourse import bacc
from concourse.bass_utils import run_bass_kernel_spmd
from concourse.masks import make_identity

import ml_dtypes

F32 = mybir.dt.float32
F32R = mybir.dt.float32r
BF16 = mybir.dt.bfloat16
AF = mybir.ActivationFunctionType
OP = mybir.AluOpType

P = 128
B, L, DIM = 4, 2048, 1024
D_STATE, D_CONV = 16, 4
D_INNER = 2048
DH = D_INNER // 2
DT_RANK = 64
MLP_HID = 4 * DIM
T = L
TH = L // 2
NC = 8
KD = DIM // P      # 8
KDH = DH // P      # 8
KH = MLP_HID // P  # 32
NT = T // 512      # 4
NTH = TH // 512    # 2
RG = [[0, 1], [2, 3], [4, 5], [6, 7]]
# scan mj groups sized to fit 3 ypsum [P,1024] tiles (6 PSUM banks),
# leaving one [P,1024] (2 banks) for the overlapped out_proj accumulation
SCAN_GROUPS = [(0, 1, 2), (3, 4, 5), (6, 7)]

_CACHE = {}


def _build():
    nc = bacc.Bacc("TRN2", target_bir_lowering=False, debug=False, num_devices=NC)
    ein = lambda n, s, d=F32: nc.dram_tensor(n, s, d, kind="ExternalInput")
    hsT = ein("hsT", (DIM, T), BF16)
    cond_c = ein("cond_c", (P, KD))
    w_ada = ein("w_ada", (DIM, 6 * DIM), BF16)
    ada_b = ein("ada_b", (1, 6 * DIM))
    nrm_w = ein("nrm_w", (P, KD))
    nrm_b = ein("nrm_b", (P, KD))
    w_inx = ein("w_inx", (DIM, DH), BF16)
    w_inz = ein("w_inz", (DIM, DH), BF16)
    conv_wt = ein("conv_wt", (P, KDH * D_CONV))
    conv_bt = ein("conv_bt", (P, KDH))
    w_xp = ein("w_xp", (DH, 96), BF16)
    w_dt = ein("w_dt", (P, DH), BF16)
    dt_bt = ein("dt_bt", (P, KDH))
    A_sc = ein("A_sc", (P, KDH * D_STATE))
    Dp_c = ein("Dp_c", (P, KDH))
    w_out = ein("w_out", (DH, DIM), BF16)        # my d-half rows only
    w_fc1 = ein("w_fc1", (DIM, MLP_HID), BF16)
    fc1_bt = ein("fc1_bt", (P, KH))
    w_fc2 = ein("w_fc2", (MLP_HID, DIM), BF16)
    fc2_bt = ein("fc2_bt", (P, KD))
    out_hs = nc.dram_tensor("out_hs", (DIM, TH), F32, kind="ExternalOutput")

    with tile.TileContext(nc) as tc, nc.allow_low_precision(
            reason="bf16 value path; rel-err budget 2e-2"):
        import contextlib
        ctx = contextlib.ExitStack()
        sing = ctx.enter_context(tc.tile_pool(name="sing", bufs=1))
        ws = ctx.enter_context(tc.tile_pool(name="ws", bufs=2))
        wk1 = ctx.enter_context(tc.tile_pool(name="wk1", bufs=1))
        wk2 = ctx.enter_context(tc.tile_pool(name="wk2", bufs=2))
        ga = ctx.enter_context(tc.tile_pool(name="ga", bufs=3))
        ps = ctx.enter_context(tc.tile_pool(name="ps", bufs=1, space="PSUM"))
        dpool = ctx.enter_context(tc.tile_pool(name="dram", bufs=1, space="DRAM"))

        def q(i, shape=(P, 1024), dt_=F32, name="q"):
            return ps.tile(list(shape), dt_, tag=f"q{i}", name=f"{name}{i}")

        # DRAM scratch
        md_dram = dpool.tile([1, 6 * DIM], F32)
        cc1_in = dpool.tile([96, T], F32)
        cc1_out = dpool.tile([96, T], F32)
        bc_dram = dpool.tile([2 * D_STATE, T], BF16)
        zs_dram = dpool.tile([P, KDH, T], BF16)
        cc2_in = dpool.tile([2, P, KD, TH], BF16)   # out_proj partials
        cc2_out = dpool.tile([P, KD, TH], BF16)     # reduced home half

        # ---- small persistents ----
        cond_sb = sing.tile([P, KD], F32)
        nc.sync.dma_start(cond_sb[:], cond_c[:])
        scond = sing.tile([P, KD], BF16)
        nc.scalar.activation(scond[:], cond_sb[:], AF.Silu)
        nwb_sb = sing.tile([P, 2 * KD], F32)
        nc.sync.dma_start(nwb_sb[:, 0:KD], nrm_w[:])
        nc.sync.dma_start(nwb_sb[:, KD:2 * KD], nrm_b[:])
        cwb_sb = sing.tile([P, KDH * D_CONV + KDH], F32)
        nc.sync.dma_start(cwb_sb[:, 0:KDH * D_CONV], conv_wt[:, :])
        nc.sync.dma_start(cwb_sb[:, KDH * D_CONV:], conv_bt[:])
        dtb_sb = sing.tile([P, KDH], F32)
        nc.sync.dma_start(dtb_sb[:], dt_bt[:])
        Asc_sb = sing.tile([P, KDH * D_STATE], F32)
        nc.sync.dma_start(Asc_sb[:], A_sc[:])
        Dp_sb = sing.tile([P, KDH], F32)
        nc.sync.dma_start(Dp_sb[:], Dp_c[:])
        fb_sb = sing.tile([P, KH + KD], F32)
        nc.sync.dma_start(fb_sb[:, 0:KH], fc1_bt[:])
        nc.sync.dma_start(fb_sb[:, KH:], fc2_bt[:])
        ident = sing.tile([P, P], BF16)
        make_identity(nc, ident[:])
        ones_bf = sing.tile([P, 1], BF16)
        nc.vector.memset(ones_bf[:], 1.0)
        ones_row = sing.tile([1, P], F32R)
        nc.vector.memset(ones_row[:].bitcast(F32), 1.0)
        eps_sb = sing.tile([1, 2], F32)
        nc.vector.memset(eps_sb[:, 0:1], 1e-5)
        nc.vector.memset(eps_sb[:, 1:2], 1e-6)

        # ---- adaLN mods (bf16) ----
        for fj in range(12):
            wada = ws.tile([P, KD, 512], BF16, tag="ws")
            nc.sync.dma_start(wada[:], w_ada[:, fj * 512:(fj + 1) * 512]
                              .rearrange("(k p) f -> p k f", p=P))
            mp = q(fj % 2, (1, 512), name="mp")
            for kj in range(KD):
                nc.tensor.matmul(mp[:], scond[:, kj:kj + 1], wada[:, kj],
                                 start=(kj == 0), stop=(kj == KD - 1))
            adab_c = ga.tile([1, 512], F32, tag="gR")
            nc.sync.dma_start(adab_c[:], ada_b[0:1, fj * 512:(fj + 1) * 512])
            mrow = ga.tile([1, 512], F32, tag="gR")
            nc.vector.tensor_tensor(mrow[:], mp[:], adab_c[:], OP.add)
            nc.sync.dma_start(md_dram[0:1, fj * 512:(fj + 1) * 512], mrow[:])
        modsT = sing.tile([P, 6 * KD], F32)
        nc.sync.dma_start(modsT[:, 0:2 * KD],
                          md_dram[0:1, 0:2 * DIM].rearrange("q (c p) -> (q p) c", p=P))
        sh_msa, sc_msa, gt_msa, sh_mlp, sc_mlp, gt_mlp = (
            modsT[:, k * KD:(k + 1) * KD] for k in range(6))
        a1 = sing.tile([P, 3 * KD], F32)   # [a1 | c1 | a2]
        tmp8 = sing.tile([P, KD], F32)
        nc.vector.tensor_scalar(tmp8[:], sc_msa, 1.0, None, OP.add)
        nc.vector.tensor_tensor(a1[:, 0:KD], nwb_sb[:, 0:KD], tmp8[:], OP.mult)
        nc.vector.tensor_tensor(a1[:, KD:2 * KD], nwb_sb[:, KD:2 * KD], tmp8[:], OP.mult)
        nc.vector.tensor_tensor(a1[:, KD:2 * KD], a1[:, KD:2 * KD], sh_msa, OP.add)
        nc.sync.dma_start(modsT[:, 2 * KD:],
                          md_dram[0:1, 2 * DIM:].rearrange("q (c p) -> (q p) c", p=P))
        nc.vector.tensor_scalar(a1[:, 2 * KD:3 * KD], sc_mlp, 1.0, None, OP.add)
        gt2 = sing.tile([P, KD], F32)
        nc.vector.tensor_tensor(gt2[:], gt_msa, gt_msa, OP.mult)
        gt_bf = sing.tile([P, 2 * KD], BF16)
        nc.vector.tensor_copy(gt_bf[:, 0:KD], gt_msa)
        nc.vector.tensor_copy(gt_bf[:, KD:2 * KD], gt2[:])

        # ---- LN1 + modulate, fused per 512-tile ----
        x_in = sing.tile([P, KD, T], BF16, tag="BIGA")
        for ft in range(NT):
            fs = slice(ft * 512, ft * 512 + 512)
            hst = ws.tile([P, KD, 512], BF16, tag="ws")
            nc.sync.dma_start(hst[:], hsT[:, fs].rearrange("(k p) t -> p k t", p=P))
            sp = q(0, (1, 512), name="sp")
            qp = q(1, (1, 512), name="qp")
            for kj in range(KD):
                nc.tensor.matmul(sp[:], ones_bf[:], hst[:, kj],
                                 start=(kj == 0), stop=(kj == KD - 1))
            for kj in range(KD):
                sq = ga.tile([P, 512], BF16, tag="gA")
                nc.scalar.activation(sq[:], hst[:, kj], AF.Square)
                nc.tensor.matmul(qp[:], ones_bf[:], sq[:],
                                 start=(kj == 0), stop=(kj == KD - 1))
            mu = ga.tile([1, 512], F32R, tag="gR")
            nc.vector.tensor_scalar(mu[:], sp[:], 1.0 / DIM, None, OP.mult)
            mq = ga.tile([1, 512], F32, tag="gR")
            nc.vector.tensor_tensor(mq[:], mu[:], mu[:], OP.mult)
            vt = ga.tile([1, 512], F32, tag="gR")
            nc.vector.scalar_tensor_tensor(vt[:], qp[:], 1.0 / DIM, mq[:],
                                           OP.mult, OP.subtract)
            nc.scalar.activation(vt[:], vt[:], AF.Sqrt, bias=eps_sb[:, 0:1])
            rs = ga.tile([1, 512], F32R, tag="gR")
            nc.vector.reciprocal(rs[:], vt[:])
            mr_pl = q(2, (P, 1024), name="mr_pl")
            nc.tensor.matmul(mr_pl[:, 0:512], ones_row[:], mu[:], start=True, stop=True)
            nc.tensor.matmul(mr_pl[:, 512:1024], ones_row[:], rs[:], start=True, stop=True)
            for kj in range(KD):
                t1 = ga.tile([P, 512], F32, tag="gA")
                nc.vector.tensor_tensor(t1[:], hst[:, kj], mr_pl[:, 0:512], OP.subtract)
                nc.vector.tensor_tensor(t1[:], t1[:], mr_pl[:, 512:1024], OP.mult)
                nc.scalar.activation(x_in[:, kj, fs], t1[:], AF.Identity,
                                     scale=a1[:, kj:kj + 1],
                                     bias=a1[:, KD + kj:KD + kj + 1])

        # ---- in_proj X + conv + silu -> xcv_bf; x_proj folded per mj ----
        xcv_bf = sing.tile([P, KDH, T], BF16, tag="XCVB")
        xdp01 = q(2, (96, 1024), name="xdp01")
        xdp23 = q(3, (96, 1024), name="xdp23")
        xdps = [xdp01[:, 0:512], xdp01[:, 512:1024],
                xdp23[:, 0:512], xdp23[:, 512:1024]]
        wxp_sb = sing.tile([P, KDH, 96], BF16)
        nc.sync.dma_start(wxp_sb[:], w_xp[:, :].rearrange("(k p) m -> p k m", p=P))
        for mj in range(KDH):
            wx = ws.tile([P, KD, P], BF16, tag="wsm", bufs=3)
            nc.sync.dma_start(wx[:], w_inx[:, mj * P:(mj + 1) * P]
                              .rearrange("(k p) m -> p k m", p=P))
            px0 = q(0, name="px")
            px1 = q(1, name="px")
            pxs = [px0, px1]
            for ft in range(NT):
                for kj in range(KD):
                    nc.tensor.matmul(pxs[ft // 2][:, (ft % 2) * 512:(ft % 2) * 512 + 512],
                                     wx[:, kj], x_in[:, kj, ft * 512:(ft + 1) * 512],
                                     start=(kj == 0), stop=(kj == KD - 1))
            xpad = wk1.tile([P, D_CONV - 1 + T], BF16, tag="xpad")
            nc.vector.memset(xpad[:, 0:D_CONV - 1], 0.0)
            for ft in range(NT):
                nc.scalar.activation(
                    xpad[:, D_CONV - 1 + ft * 512:D_CONV - 1 + (ft + 1) * 512],
                    pxs[ft // 2][:, (ft % 2) * 512:(ft % 2) * 512 + 512], AF.Copy)
            cvt = wk1.tile([P, T], BF16, tag="cvt")
            nc.vector.tensor_scalar(cvt[:], xpad[:, 0:T],
                                    cwb_sb[:, mj * D_CONV:mj * D_CONV + 1],
                                    None, OP.mult)
            for j in range(1, D_CONV):
                nc.vector.scalar_tensor_tensor(
                    cvt[:], xpad[:, j:j + T],
                    cwb_sb[:, mj * D_CONV + j:mj * D_CONV + j + 1], cvt[:],
                    OP.mult, OP.add)
            nc.scalar.activation(
                xcv_bf[:, mj], cvt[:], AF.Silu,
                bias=cwb_sb[:, KDH * D_CONV + mj:KDH * D_CONV + mj + 1])
            for ft in range(NT):
                nc.tensor.matmul(xdps[ft][:], wxp_sb[:, mj],
                                 xcv_bf[:, mj, ft * 512:(ft + 1) * 512],
                                 start=(mj == 0), stop=(mj == KDH - 1))

        # ---- x_proj partials -> DRAM + CC1: AllReduce ----
        for ft in range(NT):
            xdc = ga.tile([96, 512], F32, tag="gA")
            nc.vector.tensor_copy(xdc[:], xdps[ft][:])
            nc.sync.dma_start(cc1_in[:, ft * 512:(ft + 1) * 512], xdc[:])
        nc.gpsimd.collective_compute("AllReduce", OP.add, ins=[cc1_in[:]],
                                     outs=[cc1_out[:]], replica_groups=RG)

        # ---- dt (softplus) + u;  B/C rows -> bc_dram ----
        dtlow = wk1.tile([P, T], BF16, tag="cvt", name="dtlow")
        nc.vector.memset(dtlow[:], 0.0)
        nc.gpsimd.dma_start(dtlow[0:DT_RANK, :], cc1_out[0:DT_RANK, :])
        nc.gpsimd.dma_start(bc_dram[:], cc1_out[DT_RANK:96, :])
        wdt_sb = wk1.tile([P, DH], BF16, tag="xpad", name="wdt_sb")
        nc.sync.dma_start(wdt_sb[:], w_dt[:])
        one_col = sing.tile([P, 1], F32)
        nc.vector.memset(one_col[:], 1.0)
        dtu = sing.tile([P, 2 * KDH, T], BF16, tag="BIGA")   # [0:8]=dt, [8:16]=u

        def dt_for_mj(mj):
            # softplus(x+b) = ln(1 + exp(x+b)); exp scratch in the u-slot,
            # then u = dt * xcv overwrites it. Uses q3 (out_proj's tag, idle
            # during the tb0 scan) so it can interleave with scan groups.
            pdt = q(3, name="pd")
            for fh in range(2):
                for ft in (2 * fh, 2 * fh + 1):
                    nc.tensor.matmul(pdt[:, (ft % 2) * 512:(ft % 2) * 512 + 512],
                                     wdt_sb[:, mj * P:(mj + 1) * P],
                                     dtlow[:, ft * 512:(ft + 1) * 512],
                                     start=True, stop=True)
                for ft in (2 * fh, 2 * fh + 1):
                    nc.scalar.activation(
                        dtu[:, KDH + mj, ft * 512:(ft + 1) * 512],
                        pdt[:, (ft % 2) * 512:(ft % 2) * 512 + 512],
                        AF.Exp, bias=dtb_sb[:, mj:mj + 1])
                pdt = q(3, name="pd")
            nc.scalar.activation(dtu[:, mj], dtu[:, KDH + mj], AF.Ln, bias=one_col[:])
            nc.vector.tensor_tensor(dtu[:, KDH + mj], dtu[:, mj], xcv_bf[:, mj], OP.mult)

        # ---- in_proj Z (runs during AllReduce) -> silu -> zs_dram ----
        for mj in range(KDH):
            wz = ws.tile([P, KD, P], BF16, tag="wsm", bufs=3)
            nc.sync.dma_start(wz[:], w_inz[:, mj * P:(mj + 1) * P]
                              .rearrange("(k p) m -> p k m", p=P))
            pz0 = q(2 * (mj % 2), name="pz")
            pz1 = q(2 * (mj % 2) + 1, name="pz")
            pzs = [pz0, pz1]
            for ft in range(NT):
                for kj in range(KD):
                    nc.tensor.matmul(pzs[ft // 2][:, (ft % 2) * 512:(ft % 2) * 512 + 512],
                                     wz[:, kj], x_in[:, kj, ft * 512:(ft + 1) * 512],
                                     start=(kj == 0), stop=(kj == KD - 1))
            for ft in range(NT):
                zc = ga.tile([P, 512], BF16, tag="gA")
                nc.scalar.activation(zc[:],
                                     pzs[ft // 2][:, (ft % 2) * 512:(ft % 2) * 512 + 512],
                                     AF.Silu)
                nc.sync.dma_start(zs_dram[:, mj, ft * 512:(ft + 1) * 512], zc[:])

        # ---- scan + y + gate + out_proj partials (overlapped) ----
        hcol = sing.tile([P, KDH * D_STATE], F32)
        NBT = 2
        TB = T // NBT

        def out_proj_block(tb, ygated):
            # out partial for time block tb: [DIM, TB] = sum_mj w_out[mj] y[mj]
            for oj in range(KD):
                wo = ws.tile([P, KDH, P], BF16, tag="wsm", bufs=3)
                nc.sync.dma_start(wo[:], w_out[:, oj * P:(oj + 1) * P]
                                  .rearrange("(k p) m -> p k m", p=P))
                po = q(3, name="po")
                for kj in range(KDH):
                    for hh in range(2):
                        nc.tensor.matmul(
                            po[:, hh * 512:(hh + 1) * 512], wo[:, kj],
                            ygated[:, kj, hh * 512:(hh + 1) * 512],
                            start=(kj == 0), stop=(kj == KDH - 1))
                for hh in range(2):
                    oc = ga.tile([P, 512], BF16, tag="gA")
                    nc.scalar.activation(oc[:], po[:, hh * 512:(hh + 1) * 512], AF.Copy)
                    nc.sync.dma_start(
                        cc2_in[tb, :, oj, hh * 512:(hh + 1) * 512], oc[:])

        ym_rr = 0  # round-robin: every 3rd ym on Pool (gpsimd), rest on DVE
        for tb in range(NBT):
            ts_ = slice(tb * TB, (tb + 1) * TB)
            ygated = wk1.tile([P, KDH, TB], BF16, tag="YGD", name=f"ygated{tb}")
            for gi, grp in enumerate(SCAN_GROUPS):
                if tb == 0:
                    for mj in grp:
                        dt_for_mj(mj)
                ypsum = {mj: q(j, name="ypsum") for j, mj in enumerate(grp)}
                for mj in grp:
                    dpl = wk2.tile([P, TB], BF16, tag="bin", name="dpl")
                    nc.scalar.activation(dpl[:], xcv_bf[:, mj, ts_], AF.Identity,
                                         scale=Dp_sb[:, mj:mj + 1])
                    for hh in range(2):
                        nc.tensor.matmul(ypsum[mj][:, hh * 512:(hh + 1) * 512],
                                         ident[:], dpl[:, hh * 512:(hh + 1) * 512],
                                         start=True, stop=False)
                for n in range(D_STATE):
                    bpl = wk2.tile([P, TB], BF16, tag="bpl")
                    nc.sync.dma_start(bpl[:], bc_dram[n:n + 1, ts_].partition_broadcast(P))
                    cpl = wk2.tile([P, TB], BF16, tag="cpl")
                    nc.sync.dma_start(cpl[:], bc_dram[D_STATE + n:D_STATE + n + 1, ts_]
                                      .partition_broadcast(P))
                    for mj in grp:
                        da = wk2.tile([P, TB], F32, tag="da")
                        nc.scalar.activation(
                            da[:], dtu[:, mj, ts_], AF.Exp,
                            scale=Asc_sb[:, mj * D_STATE + n:mj * D_STATE + n + 1])
                        bin_ = wk2.tile([P, TB], BF16, tag="bin")
                        beng = nc.gpsimd if ym_rr % 5 in (1, 3) else nc.vector
                        beng.tensor_tensor(bin_[:], dtu[:, KDH + mj, ts_], bpl[:],
                                           OP.mult)
                        h = wk2.tile([P, TB], BF16, tag="h", bufs=4)
                        init = (0.0 if tb == 0
                                else hcol[:, mj * D_STATE + n:mj * D_STATE + n + 1])
                        nc.vector.tensor_tensor_scan(h[:], da[:], bin_[:], init,
                                                     OP.mult, OP.add)
                        if tb < NBT - 1:
                            nc.scalar.activation(
                                hcol[:, mj * D_STATE + n:mj * D_STATE + n + 1],
                                h[:, TB - 1:TB], AF.Copy)
                        ym = wk2.tile([P, TB], BF16, tag="ym")
                        eng = nc.gpsimd if ym_rr % 5 in (0, 2) else nc.vector
                        ym_rr += 1
                        eng.tensor_tensor(ym[:], h[:], cpl[:], OP.mult)
                        for hh in range(2):
                            nc.tensor.matmul(ypsum[mj][:, hh * 512:(hh + 1) * 512],
                                             ident[:], ym[:, hh * 512:(hh + 1) * 512],
                                             start=False, stop=(n == D_STATE - 1))
                for mj in grp:
                    zl = wk2.tile([P, TB], BF16, tag="bpl", name="zl")
                    nc.sync.dma_start(zl[:], zs_dram[:, mj, ts_])
                    nc.vector.tensor_tensor(ygated[:, mj], ypsum[mj][:], zl[:], OP.mult)
            out_proj_block(tb, ygated)
        nc.gpsimd.collective_compute("ReduceScatter", OP.add,
                                     ins=[cc2_in[:, :, 0:KD // 2, :]],
                                     outs=[cc2_out[:, 0:KD // 2, :]],
                                     replica_groups=RG)
        nc.gpsimd.collective_compute("ReduceScatter", OP.add,
                                     ins=[cc2_in[:, :, KD // 2:, :]],
                                     outs=[cc2_out[:, KD // 2:, :]],
                                     replica_groups=RG)

        # ---- gate_msa + LN2 + modulate (SBUF resident) ----
        hsm = sing.tile([P, KD, TH], F32, tag="XCVB")
        h2_bf = sing.tile([P, KD, TH], BF16, tag="ZSC")
        for ft in range(NTH):
            fs = slice(ft * 512, ft * 512 + 512)
            sp = q(0, (1, 512), name="sp2")
            qp = q(1, (1, 512), name="qp2")
            hmst = ws.tile([P, KD, 512], BF16, tag="ws")
            # per kj-half: load + gate + gated stats, so the oj 0-3 half
            # proceeds while the second ReduceScatter is still in flight
            for half in range(2):
                ks = range(half * KD // 2, (half + 1) * KD // 2)
                nc.sync.dma_start(hmst[:, half * KD // 2:(half + 1) * KD // 2],
                                  cc2_out[:, half * KD // 2:(half + 1) * KD // 2, fs])
                for kj in ks:
                    nc.vector.tensor_scalar(hsm[:, kj, fs], hmst[:, kj],
                                            gt_msa[:, kj:kj + 1], None, OP.mult)
                for kj in ks:
                    nc.tensor.matmul(sp[:], gt_bf[:, kj:kj + 1], hmst[:, kj],
                                     start=(kj == 0), stop=(kj == KD - 1))
                for kj in ks:
                    sq = ga.tile([P, 512], BF16, tag="gA")
                    nc.scalar.activation(sq[:], hmst[:, kj], AF.Square)
                    nc.tensor.matmul(qp[:], gt_bf[:, KD + kj:KD + kj + 1], sq[:],
                                     start=(kj == 0), stop=(kj == KD - 1))
            mu = ga.tile([1, 512], F32R, tag="gR")
            nc.vector.tensor_scalar(mu[:], sp[:], 1.0 / DIM, None, OP.mult)
            mq = ga.tile([1, 512], F32, tag="gR")
            nc.vector.tensor_tensor(mq[:], mu[:], mu[:], OP.mult)
            vt = ga.tile([1, 512], F32, tag="gR")
            nc.vector.scalar_tensor_tensor(vt[:], qp[:], 1.0 / DIM, mq[:],
                                           OP.mult, OP.subtract)
            nc.scalar.activation(vt[:], vt[:], AF.Sqrt, bias=eps_sb[:, 1:2])
            rs = ga.tile([1, 512], F32R, tag="gR")
            nc.vector.reciprocal(rs[:], vt[:])
            mr_pl = q(2, (P, 1024), name="mr_pl2")
            nc.tensor.matmul(mr_pl[:, 0:512], ones_row[:], mu[:], start=True, stop=True)
            nc.tensor.matmul(mr_pl[:, 512:1024], ones_row[:], rs[:], start=True, stop=True)
            for kj in range(KD):
                t1 = ga.tile([P, 512], F32, tag="gA")
                nc.vector.tensor_tensor(t1[:], hsm[:, kj, fs], mr_pl[:, 0:512],
                                        OP.subtract)
                nc.vector.tensor_tensor(t1[:], t1[:], mr_pl[:, 512:1024], OP.mult)
                nc.scalar.activation(h2_bf[:, kj, fs], t1[:], AF.Identity,
                                     scale=a1[:, 2 * KD + kj:2 * KD + kj + 1],
                                     bias=sh_mlp[:, kj:kj + 1])

        # ---- MLP (weight-stationary) ----
        hmlp = sing.tile([P, KH, TH], BF16, tag="BIGA")
        for mj in range(KH):
            wf1 = ws.tile([P, KD, P], BF16, tag="wsm", bufs=3)
            nc.sync.dma_start(wf1[:], w_fc1[:, mj * P:(mj + 1) * P]
                              .rearrange("(k p) m -> p k m", p=P))
            pm = q(mj % 2, name="pm")
            for kj in range(KD):
                for ft in range(NTH):
                    nc.tensor.matmul(pm[:, ft * 512:(ft + 1) * 512],
                                     wf1[:, kj], h2_bf[:, kj, ft * 512:(ft + 1) * 512],
                                     start=(kj == 0), stop=(kj == KD - 1))
            for ft in range(NTH):
                nc.scalar.activation(hmlp[:, mj, ft * 512:(ft + 1) * 512],
                                     pm[:, ft * 512:(ft + 1) * 512],
                                     AF.Gelu_apprx_tanh, bias=fb_sb[:, mj:mj + 1])
        for oj in range(KD):
            wf2 = ws.tile([P, KH, P], BF16, tag="ws")
            nc.sync.dma_start(wf2[:], w_fc2[:, oj * P:(oj + 1) * P]
                              .rearrange("(k p) m -> p k m", p=P))
            pf = q(2 + oj % 2, name="pf")
            for kj in range(KH):
                for ft in range(NTH):
                    nc.tensor.matmul(pf[:, ft * 512:(ft + 1) * 512],
                                     wf2[:, kj], hmlp[:, kj, ft * 512:(ft + 1) * 512],
                                     start=(kj == 0), stop=(kj == KH - 1))
            for ft in range(NTH):
                fs = slice(ft * 512, ft * 512 + 512)
                t1 = ga.tile([P, 512], F32, tag="gA")
                nc.vector.tensor_scalar(t1[:], pf[:, fs], fb_sb[:, KH + oj:KH + oj + 1],
                                        gt_mlp[:, oj:oj + 1], OP.add, OP.mult)
                nc.vector.tensor_tensor(t1[:], t1[:], hsm[:, oj, fs], OP.add)
                nc.sync.dma_start(
                    out_hs[:, :].rearrange("(k p) t -> p k t", p=P)[:, oj, fs], t1[:])
        ctx.close()
    nc.compile()
    return nc


def _prep_inputs(kw):
    f32 = np.float32
    bf16 = ml_dtypes.bfloat16
    g = lambda k: np.asarray(kw[k], f32)
    hs, cond = g("hidden_states"), g("cond")
    in_w = g("in_proj_w")
    A = -np.exp(g("A_log"))
    out_w = g("out_proj_w")

    def colchunks(v):
        return np.ascontiguousarray(v.reshape(-1, P).T)

    adaT = np.ascontiguousarray(g("ada_w").T).astype(bf16)
    fc1T = np.ascontiguousarray(g("fc1_w").T).astype(bf16)
    fc2T = np.ascontiguousarray(g("fc2_w").T).astype(bf16)
    in_maps = []
    for c in range(NC):
        b, i = c // 2, c % 2
        dsl = slice(i * DH, (i + 1) * DH)
        A_h = A[dsl]
        A_pack = np.zeros((P, KDH * D_STATE), f32)
        for mj in range(KDH):
            A_pack[:, mj * D_STATE:(mj + 1) * D_STATE] = A_h[mj * P:(mj + 1) * P]
        cw = g("conv_w")[dsl, 0, :].reshape(KDH, P, D_CONV).transpose(1, 0, 2)
        in_maps.append({
            "hsT": np.ascontiguousarray(hs[b].T).astype(bf16),
            "cond_c": colchunks(cond[b]),
            "w_ada": adaT,
            "ada_b": g("ada_b").reshape(1, -1),
            "nrm_w": colchunks(g("norm_w")),
            "nrm_b": colchunks(g("norm_b")),
            "w_inx": np.ascontiguousarray(in_w[dsl].T).astype(bf16),
            "w_inz": np.ascontiguousarray(in_w[D_INNER:][dsl].T).astype(bf16),
            "conv_wt": np.ascontiguousarray(cw.reshape(P, KDH * D_CONV)),
            "conv_bt": colchunks(g("conv_b")[dsl]),
            "w_xp": np.ascontiguousarray(g("x_proj_w")[:, dsl].T).astype(bf16),
            "w_dt": np.concatenate([g("dt_proj_w")[dsl].T,
                                    np.zeros((P - DT_RANK, DH), f32)], 0).astype(bf16),
            "dt_bt": colchunks(g("dt_proj_b")[dsl]),
            "A_sc": A_pack,
            "Dp_c": colchunks(g("Dp")[dsl]),
            "w_out": np.ascontiguousarray(out_w[:, dsl].T).astype(bf16),
            "w_fc1": fc1T,
            "fc1_bt": colchunks(g("fc1_b")),
            "w_fc2": fc2T,
            "fc2_bt": colchunks(g("fc2_b")),
        })
    return in_maps


def kernel(**inputs):
    if "nc" not in _CACHE:
        _CACHE["nc"] = _build()
    nc = _CACHE["nc"]
    in_maps = _prep_inputs(inputs)
    res = run_bass_kernel_spmd(nc, in_maps, list(range(NC)))
    hs_out = np.empty((B, L, DIM), np.float32)
    for c in range(NC):
        b, i = c // 2, c % 2
        hs_out[b, i * TH:(i + 1) * TH, :] = res.results[c]["out_hs"].T
    residual = np.asarray(inputs["hidden_states"], np.float32)
    return hs_out, residual
